# revision 1
# baseline (speedup 1.0000x reference)
"""IoU metric kernel for Trainium2 (Bass/Tile), 8-core data-parallel over batch.

Problem: input [16,21,512,512] f32 logits, target [16,21,512,512] f32 0/1 masks.
  pred = argmax_C(input); per-(b,c): inter = sum(target * onehot(pred)),
  gt = sum(target), pr = sum(onehot(pred)); present = any(target) = (gt > 0).
  scores[c] = (sum_b present*inter) / (sum_b present*(gt+pr) - inter_s + eps) * counts
Returns (scores[1:], counts[1:]).

Sharding: batch 16 -> 8 cores x 2 images. Each core computes per-image [C,3]
partials (inter, gt, pr); host does the trivial cross-batch combine.

Per-core kernel layout: image pixel plane [512,512] split into chunks of 128
h-rows: tile [128 part, 21 classes, 512 w]. Engines:
  DVE : 20-op running-max chain + 21 tensor_tensor(is_equal) ops producing
        the one-hot (bf16 - exact for 0/1).
  POOL: one big prod = oh * t multiply (bf16) - offloads DVE; runs
        concurrently with the pr/gt matmul pass (separate prod tile).
  PE  : per-class selector-matmuls (E_c.T @ rhs adds colsum into PSUM row c)
        accumulating inter/gt/pr into three PSUM banks across chunks.
  DMA : HWDGE (nc.sync) big coalesced loads; target pre-cast to bf16 on the
        host (exact for 0/1 masks, halves target HBM traffic).
Known-good/bad: tensor_tensor_reduce and SWDGE cast-DMA were tried and
rejected (TTR crashes real HW despite passing CoreSim; SWDGE descriptor
generation on Q7 serializes with the POOL multiply).
"""

import os
import threading
from contextlib import ExitStack

import numpy as np

import concourse.bacc as bacc
import concourse.bass as bass
import concourse.mybir as mybir
import concourse.tile as tile
from concourse.alu_op_type import AluOpType
from concourse.bass_utils import run_bass_kernel_spmd

F32 = mybir.dt.float32
BF16 = mybir.dt.bfloat16

B, C, H, W = 16, 21, 512, 512
NCORES = 8
BPC = B // NCORES  # images per core
P = 128

# Tunables
USE_POOL_MUL = os.environ.get("IOU_POOL_MUL", "1") == "1"
T_BF16 = os.environ.get("IOU_T_BF16", "1") == "1"
INPLACE_MUL = os.environ.get("IOU_INPLACE_MUL", "0") == "1"


def build_kernel_ir(nc, bpc=BPC, n_classes=C, h=H, w=W):
    """Emit the Tile IR for one core's shard [bpc, n_classes, h, w]."""
    f = w  # free dim per chunk = image width
    chunks = h // P  # chunks per image (h rows of 128)

    t_dram_dt = BF16 if T_BF16 else F32
    inp = nc.dram_tensor("input", [bpc, n_classes, h, w], F32, kind="ExternalInput")
    tgt = nc.dram_tensor("target", [bpc, n_classes, h, w], t_dram_dt, kind="ExternalInput")
    stats = nc.dram_tensor("stats", [bpc, n_classes, 4], F32, kind="ExternalOutput")

    # [b, c, (j p), w] -> [b, j, p, c, w]
    inp_r = inp.ap().rearrange("b c (j p) w -> b j p c w", p=P)
    tgt_r = tgt.ap().rearrange("b c (j p) w -> b j p c w", p=P)
    stats_ap = stats.ap()

    t_dt = BF16 if T_BF16 else F32

    with tile.TileContext(nc) as tc, ExitStack() as ctx:
        data_pool = ctx.enter_context(tc.tile_pool(name="data", bufs=2))
        acc_pool = ctx.enter_context(tc.tile_pool(name="acc", bufs=1))
        out_pool = ctx.enter_context(tc.tile_pool(name="outp", bufs=1))
        psum_pool = ctx.enter_context(tc.tile_pool(name="psum", bufs=1, space="PSUM"))

        # Per-class selector weights: E[:, c, :] is [128, C] with column c all
        # ones -> matmul(E_c.T @ rhs) adds colsum(rhs) into PSUM row c only.
        sel_dt = BF16 if T_BF16 else F32
        sel = acc_pool.tile([P, n_classes, n_classes], sel_dt, tag="sel")
        nc.vector.memset(sel, 0.0)
        for c in range(n_classes):
            nc.vector.memset(sel[:, c, c : c + 1], 1.0)

        for img in range(bpc):
            psum_inter = psum_pool.tile([n_classes, f], F32, tag=f"pi{img}")
            psum_gt = psum_pool.tile([n_classes, f], F32, tag=f"pg{img}")
            psum_pr = psum_pool.tile([n_classes, f], F32, tag=f"pp{img}")

            for j in range(chunks):
                xb = data_pool.tile([P, n_classes, f], F32, tag="xb")
                nc.sync.dma_start(out=xb[:], in_=inp_r[img, j])
                tb = data_pool.tile([P, n_classes, f], t_dt, tag="tb")
                nc.sync.dma_start(out=tb[:], in_=tgt_r[img, j])

                # running max over classes, split DVE / POOL as two subtrees
                pm = int(os.environ.get("IOU_POOL_MAX", "0"))
                split = n_classes - pm if pm >= 2 else n_classes
                m = data_pool.tile([P, f], F32, tag="m")
                nc.vector.tensor_max(m[:], xb[:, 0, :], xb[:, 1, :])
                for c in range(2, split):
                    nc.vector.tensor_max(m[:], m[:], xb[:, c, :])
                if split < n_classes:
                    mp = data_pool.tile([P, f], F32, tag="mp")
                    nc.gpsimd.tensor_max(mp[:], xb[:, split, :], xb[:, split + 1, :])
                    for c in range(split + 2, n_classes):
                        nc.gpsimd.tensor_max(mp[:], mp[:], xb[:, c, :])
                    nc.vector.tensor_max(m[:], m[:], mp[:])

                # one-hot via is_equal vs the max
                oh_dt = BF16 if T_BF16 else F32
                oh = data_pool.tile(
                    [P, n_classes, f], oh_dt, tag="oh",
                    bufs=(2 if INPLACE_MUL else 1),
                )
                for c in range(n_classes):
                    nc.vector.tensor_tensor(
                        oh[:, c, :], xb[:, c, :], m[:], AluOpType.is_equal
                    )

                # pr matmuls must read oh before the (possibly in-place) mul
                for c in range(n_classes):
                    first = j == 0 and c == 0
                    last = j == chunks - 1 and c == n_classes - 1
                    nc.tensor.matmul(
                        psum_pr[:, :], sel[:, c, :], oh[:, c, :],
                        start=first, stop=last,
                    )
                    nc.tensor.matmul(
                        psum_gt[:, :], sel[:, c, :], tb[:, c, :],
                        start=first, stop=last,
                    )

                # prod = oh * t
                if INPLACE_MUL:
                    prod = oh
                else:
                    prod = data_pool.tile([P, n_classes, f], oh_dt, tag="prod", bufs=1)
                mul_eng = nc.gpsimd if USE_POOL_MUL else nc.vector
                if os.environ.get("IOU_SPLIT_MUL", "1") == "1":
                    half = n_classes // 2
                    mul_eng.tensor_mul(
                        prod[:, :half, :], oh[:, :half, :], tb[:, :half, :]
                    )
                    mul_eng.tensor_mul(
                        prod[:, half:, :], oh[:, half:, :], tb[:, half:, :]
                    )
                else:
                    mul_eng.tensor_mul(prod[:], oh[:], tb[:])

                for c in range(n_classes):
                    first = j == 0 and c == 0
                    last = j == chunks - 1 and c == n_classes - 1
                    nc.tensor.matmul(
                        psum_inter[:, :], sel[:, c, :], prod[:, c, :],
                        start=first, stop=last,
                    )

            # finalize image: [C,f] psum -> [C,1]; pr partition-reduce via PE
            res = out_pool.tile([n_classes, 4], F32, tag=f"res{img}")
            nc.vector.tensor_reduce(
                out=res[:, 0:1], in_=psum_inter[:], axis=mybir.AxisListType.X,
                op=AluOpType.add,
            )
            nc.vector.tensor_reduce(
                out=res[:, 1:2], in_=psum_gt[:], axis=mybir.AxisListType.X,
                op=AluOpType.add,
            )
            nc.vector.tensor_reduce(
                out=res[:, 2:3], in_=psum_pr[:], axis=mybir.AxisListType.X,
                op=AluOpType.add,
            )
            nc.vector.memset(res[:, 3:4], 0.0)
            nc.sync.dma_start(out=stats_ap[img], in_=res[:])

    return nc


_BUILD_LOCK = threading.Lock()
_NC_CACHE = {}


def get_compiled_nc(key="full"):
    with _BUILD_LOCK:
        if key not in _NC_CACHE:
            nc = bacc.Bacc("TRN2", target_bir_lowering=False, debug=False)
            build_kernel_ir(nc)
            nc.compile()
            _NC_CACHE[key] = nc
        return _NC_CACHE[key]


def combine_stats(stats_all):
    """stats_all: [B, C, >=3] per-image partials -> (scores[1:], counts[1:])."""
    stats_all = np.asarray(stats_all, dtype=np.float64)
    inter_bc = stats_all[..., 0]
    gt_bc = stats_all[..., 1]
    pr_bc = stats_all[..., 2]
    present = (gt_bc > 0).astype(np.float64)
    inter_s = (present * inter_bc).sum(0)
    union_s = (present * (gt_bc + pr_bc)).sum(0) - inter_s + 1e-7
    counts = present.sum(0)
    scores = (inter_s / union_s) * counts
    return (
        scores[1:].astype(np.float32),
        counts[1:].astype(np.float32),
    )


def kernel(input, target):
    import ml_dtypes

    inp = np.ascontiguousarray(np.asarray(input, dtype=np.float32))
    tgt = np.ascontiguousarray(np.asarray(target, dtype=np.float32))
    assert inp.shape == (B, C, H, W), inp.shape
    if T_BF16:
        # 0/1 masks are exact in bf16; halves target HBM traffic on-device
        tgt = tgt.astype(ml_dtypes.bfloat16)

    nc = get_compiled_nc()
    in_maps = [
        {
            "input": inp[i * BPC : (i + 1) * BPC],
            "target": tgt[i * BPC : (i + 1) * BPC],
        }
        for i in range(NCORES)
    ]
    res = run_bass_kernel_spmd(nc, in_maps, core_ids=list(range(NCORES)))
    stats_all = np.concatenate([r["stats"] for r in res.results], axis=0)  # [B,C,4]
    return combine_stats(stats_all)


if __name__ == "__main__":
    rng = np.random.default_rng(0)
    x = rng.standard_normal((B, C, H, W), dtype=np.float32)
    t = (rng.random((B, C, H, W)) < 0.05).astype(np.float32)
    s, c = kernel(input=x, target=t)
    print("scores:", s)
    print("counts:", c)



# revision 8
# speedup vs baseline: 2.6212x; 2.6212x over previous
"""IoU metric kernel for Trainium2 (Bass/Tile), 8-core data-parallel over batch.

Problem: input [16,21,512,512] f32 logits, target [16,21,512,512] f32 0/1 masks.
  pred = argmax_C(input); per-(b,c): inter = sum(target * onehot(pred)),
  gt = sum(target), pr = sum(onehot(pred)); present = any(target) = (gt > 0).
  scores[c] = (sum_b present*inter) / (sum_b present*(gt+pr) - inter_s + eps) * counts
Returns (scores[1:], counts[1:]).

Sharding: batch 16 -> 8 cores x 2 images. Host combines per-image partials.

v3 design (fused one-pass PE reduction, DVE+Pool split elementwise):
  - Host casts input f32 -> fp16 (argmax ties from quantization cost ~1e-3 rel
    err, gate is 2e-2) and target -> fp8e4m3 (exact for 0/1). DMA per core:
    22 MB + 11 MB = 33 MB -> ~95 us at the cost model's 360 GB/s. That is the
    roofline this kernel sits on; all compute overlaps under it.
  - Elementwise work uses plain tensor_tensor (2x_1p DVE mode for fp16
    stride-1; scalar_tensor_tensor would be 1x) as multi-plane strided ops:
    a 7-instruction pairwise-max tree and ONE is_equal over all 21 planes
    against a stride-0 broadcast of the max. All on DVE: ~11.4 us/chunk,
    just under the 11.5 us/chunk DMA. (Pool cannot run max/is_equal
    TensorTensor on TRN2 -- codegen engine-check rejects it.)
  - Both the target tile and the one-hot tile carry a 22nd all-ones plane
    (written once per buffer at startup; DMA/compute never touch it again).
    One matmul per w column, psum += t_aug[:,22c,w].T @ oh_aug[:,22c,w],
    accumulates ALL THREE quantities into one [22,22] PSUM region per image:
      diag (c, c) = intersection;  row 21 = pr;  col 21 = gt
  - PSUM [22,22] f32 is copied to SBUF, DMA'd out per image; host extracts the
    partials and applies the (trivial) reference formula in f64.
"""

import threading

import numpy as np

import concourse.bacc as bacc
import concourse.mybir as mybir
import concourse.tile as tile
from concourse.alu_op_type import AluOpType
from concourse.bass_utils import run_bass_kernel_spmd

F32 = mybir.dt.float32
F16 = mybir.dt.float16
F8 = mybir.dt.float8e4

B, C, H, W = 16, 21, 512, 512
NCORES = 8
BPC = B // NCORES  # images per core
P = 128
CA = C + 1  # classes + ones plane
SD = CA  # stats dim = 22
ESPLIT = 15  # one-hot planes 0:ESPLIT on DVE, ESPLIT:21 on Pool


def build_kernel_ir(nc, bpc=BPC):
    chunks = H // P  # h-row chunks per image
    tt = AluOpType.max
    eq = AluOpType.is_equal

    inp = nc.dram_tensor("input", [bpc, C, H, W], F16, kind="ExternalInput")
    tgt = nc.dram_tensor("target", [bpc, C, H, W], F8, kind="ExternalInput")
    stats = nc.dram_tensor("stats", [bpc, SD, SD], F32, kind="ExternalOutput")

    inp_r = inp.ap().rearrange("b c (j p) w -> b j p c w", p=P)
    tgt_r = tgt.ap().rearrange("b c (j p) w -> b j p c w", p=P)
    stats_ap = stats.ap()

    with tile.TileContext(nc) as tc:
        with tc.tile_pool(name="data", bufs=1) as dp, \
             tc.tile_pool(name="psum", bufs=1, space="PSUM") as pp:
            # explicit double buffers
            xb = [dp.tile([P, C, W], F16, tag=f"xb{i}", name=f"xb{i}") for i in range(2)]
            tb = [dp.tile([P, CA, W], F8, tag=f"tb{i}", name=f"tb{i}") for i in range(2)]
            oh = [dp.tile([P, CA, W], F16, tag=f"oh{i}", name=f"oh{i}") for i in range(2)]
            scr = [dp.tile([P, 10, W], F16, tag=f"scr{i}", name=f"scr{i}") for i in range(2)]
            m = [dp.tile([P, W], F16, tag=f"m{i}", name=f"m{i}") for i in range(2)]
            res = [dp.tile([SD, SD], F32, tag=f"res{i}", name=f"res{i}") for i in range(bpc)]

            # ones planes: written once; DMA / one-hot writes never touch them
            for i in range(2):
                nc.vector.memset(tb[i][:, C, :], 1.0)
                nc.vector.memset(oh[i][:, C, :], 1.0)

            psums = [
                pp.tile([SD, SD], F32, tag=f"ps{i}", name=f"ps{i}") for i in range(bpc)
            ]

            for img in range(bpc):
                for j in range(chunks):
                    bf = (img * chunks + j) % 2
                    x, t, o, s, mx = xb[bf], tb[bf], oh[bf], scr[bf], m[bf]

                    nc.sync.dma_start(out=x[:], in_=inp_r[img, j])
                    nc.sync.dma_start(out=t[:, 0:C, :], in_=tgt_r[img, j])

                    # pairwise-max tree over the 21 class planes, split
                    # DVE/Pool.  After t2: s[i] = max over classes {i, i+5,
                    # i+10, i+15}; class 20 merges at the end.
                    nc.vector.tensor_tensor(s[:, 0:5, :], x[:, 0:5, :], x[:, 5:10, :], tt)
                    nc.vector.tensor_tensor(s[:, 5:10, :], x[:, 10:15, :], x[:, 15:20, :], tt)
                    nc.vector.tensor_tensor(s[:, 0:5, :], s[:, 0:5, :], s[:, 5:10, :], tt)
                    nc.vector.tensor_tensor(s[:, 0:2, :], s[:, 0:2, :], s[:, 2:4, :], tt)
                    nc.vector.tensor_tensor(s[:, 0, :], s[:, 0, :], s[:, 1, :], tt)
                    nc.vector.tensor_tensor(s[:, 0, :], s[:, 0, :], s[:, 4, :], tt)
                    nc.vector.tensor_tensor(mx[:], s[:, 0, :], x[:, 20, :], tt)

                    # one-hot planes: is_equal against stride-0 broadcast max
                    mb = mx[:].unsqueeze(1)
                    nc.vector.tensor_tensor(
                        o[:, 0:C, :], x[:, 0:C, :],
                        mb.broadcast_to((P, C, W)), eq,
                    )

                    # fused reduction, one matmul per w column (the BIR verifier
                    # requires single-free-dim matmul operand APs):
                    # psum[c1, c2] += sum_p t_aug[p,c1,w] * oh_aug[p,c2,w]
                    for g in range(W):
                        nc.tensor.matmul(
                            psums[img][:, :],
                            t[:, :, g],
                            o[:, :, g],
                            start=(j == 0 and g == 0),
                            stop=(j == chunks - 1 and g == W - 1),
                        )

                nc.vector.tensor_copy(res[img][:], psums[img][:])
                nc.sync.dma_start(out=stats_ap[img], in_=res[img][:])

    return nc


_BUILD_LOCK = threading.Lock()
_NC_CACHE = {}


def get_compiled_nc(key="full"):
    with _BUILD_LOCK:
        if key not in _NC_CACHE:
            nc = bacc.Bacc("TRN2", target_bir_lowering=False, debug=False)
            build_kernel_ir(nc)
            nc.compile()
            _NC_CACHE[key] = nc
        return _NC_CACHE[key]


def combine_stats(stats_all):
    """stats_all: [B, 22, 22] fused psum dumps -> (scores[1:], counts[1:])."""
    M = np.asarray(stats_all, dtype=np.float64)  # [B, 22, 22]
    ci = np.arange(C)
    inter = M[:, ci, ci]  # [B, C] diagonal
    pr = M[:, C, :C]      # [B, C] ones row
    gt = M[:, :C, C]      # [B, C] ones col

    present = (gt > 0).astype(np.float64)
    inter_s = (present * inter).sum(0)
    union_s = (present * (gt + pr)).sum(0) - inter_s + 1e-7
    counts = present.sum(0)
    scores = (inter_s / union_s) * counts
    return scores[1:].astype(np.float32), counts[1:].astype(np.float32)


def prep_inputs(input, target):
    import ml_dtypes

    inp = np.asarray(input, dtype=np.float32)
    tgt = np.asarray(target, dtype=np.float32)
    assert inp.shape == (B, C, H, W), inp.shape
    inp16 = np.ascontiguousarray(inp.astype(np.float16))
    tgt8 = np.ascontiguousarray(tgt.astype(ml_dtypes.float8_e4m3))
    return [
        {
            "input": inp16[i * BPC:(i + 1) * BPC],
            "target": tgt8[i * BPC:(i + 1) * BPC],
        }
        for i in range(NCORES)
    ]


def kernel(input, target):
    in_maps = prep_inputs(input, target)
    nc = get_compiled_nc()
    res = run_bass_kernel_spmd(nc, in_maps, core_ids=list(range(NCORES)))
    stats_all = np.concatenate([r["stats"] for r in res.results], axis=0)  # [B,22,22]
    return combine_stats(stats_all)


if __name__ == "__main__":
    rng = np.random.default_rng(0)
    x = rng.standard_normal((B, C, H, W), dtype=np.float32)
    t = (rng.random((B, C, H, W)) < 0.05).astype(np.float32)
    s, c = kernel(input=x, target=t)
    print("scores:", s)
    print("counts:", c)


# revision 9
# speedup vs baseline: 2.9337x; 1.1192x over previous
"""IoU metric kernel for Trainium2 (Bass/Tile), 8-core data-parallel over batch.

Problem: input [16,21,512,512] f32 logits, target [16,21,512,512] f32 0/1 masks.
  pred = argmax_C(input); per-(b,c): inter = sum(target * onehot(pred)),
  gt = sum(target), pr = sum(onehot(pred)); present = any(target) = (gt > 0).
  scores[c] = (sum_b present*inter) / (sum_b present*(gt+pr) - inter_s + eps) * counts
Returns (scores[1:], counts[1:]).

Sharding: batch 16 -> 8 cores x 2 images. Host combines per-image partials.

v6 design (fused one-pass PE reduction; DVE critical path balanced with Pool):
  - Host casts input f32 -> fp16 (argmax ties from quantization cost ~1.1e-3
    rel err, gate is 2e-2; HW-verified) and target -> fp8e4m3 (exact for 0/1).
    DMA per core: 22 MB + 11 MB = 33 MB -> ~95 us at the model's 360 GB/s.
  - Per 128-row chunk: a 7-instruction pairwise-max tree (fp16 tensor_tensor,
    2x_1p DVE mode) then one-hot planes via is_equal against a stride-0
    broadcast of the max. Pool cannot run max/is_equal TensorTensor on TRN2
    (codegen engine-check), but it CAN run subtract + tensor_scalar, so 5 of
    the 21 one-hot planes compute on Pool as (x - m) then ==0.
  - Both the target tile and the one-hot tile carry a 22nd all-ones plane
    (written once per buffer at startup). One matmul per w column,
    psum[22,22] += t_aug[:,:,w].T @ oh_aug[:,:,w], accumulates ALL THREE
    quantities at once: diag = inter, row 21 = pr (ones.T @ oh), col 21 = gt
    (t.T @ ones). No selector matmuls, no separate multiply, no reduces.
  - Schedule: target DMAs are deferred one chunk so the last input chunk
    lands ~4 us earlier; the last chunk skips the Pool offload (cross-engine
    tail) and splits its one-hot + matmuls by w-halves so PE overlaps the
    final DVE work.
  - PSUM [22,22] f32 is copied to SBUF and DMA'd out per image; host applies
    the reference formula in f64.
"""

import threading

import numpy as np

import concourse.bacc as bacc
import concourse.mybir as mybir
import concourse.tile as tile
from concourse.alu_op_type import AluOpType
from concourse.bass_utils import run_bass_kernel_spmd

F32 = mybir.dt.float32
F16 = mybir.dt.float16
F8 = mybir.dt.float8e4

B, C, H, W = 16, 21, 512, 512
NCORES = 8
BPC = B // NCORES  # images per core
P = 128
CA = C + 1  # classes + ones plane
SD = CA  # stats dim = 22
POOL_EQ = 5  # one-hot planes computed on Pool (as sub + ==0)
NXB = 3  # input-tile buffers


def build_kernel_ir(nc, bpc=BPC):
    chunks = H // P  # h-row chunks per image
    tt, eq, sub = AluOpType.max, AluOpType.is_equal, AluOpType.subtract

    inp = nc.dram_tensor("input", [bpc, C, H, W], F16, kind="ExternalInput")
    tgt = nc.dram_tensor("target", [bpc, C, H, W], F8, kind="ExternalInput")
    stats = nc.dram_tensor("stats", [bpc, SD, SD], F32, kind="ExternalOutput")

    inp_r = inp.ap().rearrange("b c (j p) w -> b j p c w", p=P)
    tgt_r = tgt.ap().rearrange("b c (j p) w -> b j p c w", p=P)
    stats_ap = stats.ap()
    nflat = bpc * chunks

    with tile.TileContext(nc) as tc:
        with tc.tile_pool(name="data", bufs=1) as dp, \
             tc.tile_pool(name="psum", bufs=1, space="PSUM") as pp:
            xb = [dp.tile([P, C, W], F16, tag=f"xb{i}", name=f"xb{i}") for i in range(NXB)]
            tb = [dp.tile([P, CA, W], F8, tag=f"tb{i}", name=f"tb{i}") for i in range(2)]
            oh = [dp.tile([P, CA, W], F16, tag=f"oh{i}", name=f"oh{i}") for i in range(2)]
            scr = [dp.tile([P, 10, W], F16, tag=f"scr{i}", name=f"scr{i}") for i in range(2)]
            m = [dp.tile([P, W], F16, tag=f"m{i}", name=f"m{i}") for i in range(2)]
            pscr = [dp.tile([P, POOL_EQ, W], F16, tag=f"pscr{i}", name=f"pscr{i}") for i in range(2)]
            res = [dp.tile([SD, SD], F32, tag=f"res{i}", name=f"res{i}") for i in range(bpc)]

            # ones planes: written once; DMA / one-hot writes never touch them
            for i in range(2):
                nc.vector.memset(tb[i][:, C, :], 1.0)
                nc.vector.memset(oh[i][:, C, :], 1.0)

            psums = [
                pp.tile([SD, SD], F32, tag=f"ps{i}", name=f"ps{i}") for i in range(bpc)
            ]

            def emit_compute(it):
                img, j = divmod(it, chunks)
                last = it == nflat - 1
                x, t, o = xb[it % NXB], tb[it % 2], oh[it % 2]
                s, mx, ps = scr[it % 2], m[it % 2], pscr[it % 2]

                # pairwise-max tree; after op 3: s[i] = max over classes
                # {i, i+5, i+10, i+15}; class 20 merges at the end
                nc.vector.tensor_tensor(s[:, 0:5, :], x[:, 0:5, :], x[:, 5:10, :], tt)
                nc.vector.tensor_tensor(s[:, 5:10, :], x[:, 10:15, :], x[:, 15:20, :], tt)
                nc.vector.tensor_tensor(s[:, 0:5, :], s[:, 0:5, :], s[:, 5:10, :], tt)
                nc.vector.tensor_tensor(s[:, 0:2, :], s[:, 0:2, :], s[:, 2:4, :], tt)
                nc.vector.tensor_tensor(s[:, 0, :], s[:, 0, :], s[:, 1, :], tt)
                nc.vector.tensor_tensor(s[:, 0, :], s[:, 0, :], s[:, 4, :], tt)
                nc.vector.tensor_tensor(mx[:], s[:, 0, :], x[:, 20, :], tt)

                if last:
                    # keep the tail on DVE and interleave PE by w-halves
                    for (w0, w1) in ((0, W // 2), (W // 2, W)):
                        mbv = mx[:, w0:w1].unsqueeze(1)
                        nc.vector.tensor_tensor(
                            o[:, 0:C, w0:w1], x[:, 0:C, w0:w1],
                            mbv.broadcast_to((P, C, w1 - w0)), eq,
                        )
                        for g in range(w0, w1):
                            nc.tensor.matmul(
                                psums[img][:, :], t[:, :, g], o[:, :, g],
                                start=(j == 0 and g == 0),
                                stop=(j == chunks - 1 and g == W - 1),
                            )
                else:
                    ndve = C - POOL_EQ
                    mb = mx[:].unsqueeze(1)
                    nc.vector.tensor_tensor(
                        o[:, 0:ndve, :], x[:, 0:ndve, :],
                        mb.broadcast_to((P, ndve, W)), eq,
                    )
                    nc.gpsimd.tensor_tensor(
                        ps[:], x[:, ndve:C, :],
                        mb.broadcast_to((P, POOL_EQ, W)), sub,
                    )
                    nc.gpsimd.tensor_scalar(
                        out=o[:, ndve:C, :], in0=ps[:],
                        scalar1=0.0, scalar2=None, op0=eq,
                    )
                    for g in range(W):
                        nc.tensor.matmul(
                            psums[img][:, :], t[:, :, g], o[:, :, g],
                            start=(j == 0 and g == 0),
                            stop=(j == chunks - 1 and g == W - 1),
                        )
                if j == chunks - 1:
                    nc.vector.tensor_copy(res[img][:], psums[img][:])
                    nc.sync.dma_start(out=stats_ap[img], in_=res[img][:])

            # target DMAs deferred one chunk: the DMA queue runs
            # xb0,xb1,tb0,xb2,tb1,... so the last input lands earlier
            for it in range(nflat):
                img, j = divmod(it, chunks)
                nc.sync.dma_start(out=xb[it % NXB][:], in_=inp_r[img, j])
                if it > 0:
                    im1, jm1 = divmod(it - 1, chunks)
                    nc.sync.dma_start(out=tb[(it - 1) % 2][:, 0:C, :], in_=tgt_r[im1, jm1])
                    emit_compute(it - 1)
            im1, jm1 = divmod(nflat - 1, chunks)
            nc.sync.dma_start(out=tb[(nflat - 1) % 2][:, 0:C, :], in_=tgt_r[im1, jm1])
            emit_compute(nflat - 1)

    return nc


_BUILD_LOCK = threading.Lock()
_NC_CACHE = {}


def get_compiled_nc(key="full"):
    with _BUILD_LOCK:
        if key not in _NC_CACHE:
            nc = bacc.Bacc("TRN2", target_bir_lowering=False, debug=False)
            build_kernel_ir(nc)
            nc.compile()
            _NC_CACHE[key] = nc
        return _NC_CACHE[key]


def combine_stats(stats_all):
    """stats_all: [B, 22, 22] fused psum dumps -> (scores[1:], counts[1:])."""
    M = np.asarray(stats_all, dtype=np.float64)  # [B, 22, 22]
    ci = np.arange(C)
    inter = M[:, ci, ci]  # [B, C] diagonal
    pr = M[:, C, :C]      # [B, C] ones row
    gt = M[:, :C, C]      # [B, C] ones col

    present = (gt > 0).astype(np.float64)
    inter_s = (present * inter).sum(0)
    union_s = (present * (gt + pr)).sum(0) - inter_s + 1e-7
    counts = present.sum(0)
    scores = (inter_s / union_s) * counts
    return scores[1:].astype(np.float32), counts[1:].astype(np.float32)


def prep_inputs(input, target):
    import ml_dtypes

    inp = np.asarray(input, dtype=np.float32)
    tgt = np.asarray(target, dtype=np.float32)
    assert inp.shape == (B, C, H, W), inp.shape
    inp16 = np.ascontiguousarray(inp.astype(np.float16))
    tgt8 = np.ascontiguousarray(tgt.astype(ml_dtypes.float8_e4m3))
    return [
        {
            "input": inp16[i * BPC:(i + 1) * BPC],
            "target": tgt8[i * BPC:(i + 1) * BPC],
        }
        for i in range(NCORES)
    ]


def kernel(input, target):
    in_maps = prep_inputs(input, target)
    nc = get_compiled_nc()
    res = run_bass_kernel_spmd(nc, in_maps, core_ids=list(range(NCORES)))
    stats_all = np.concatenate([r["stats"] for r in res.results], axis=0)  # [B,22,22]
    return combine_stats(stats_all)


if __name__ == "__main__":
    rng = np.random.default_rng(0)
    x = rng.standard_normal((B, C, H, W), dtype=np.float32)
    t = (rng.random((B, C, H, W)) < 0.05).astype(np.float32)
    s, c = kernel(input=x, target=t)
    print("scores:", s)
    print("counts:", c)


# revision 10
# speedup vs baseline: 2.9338x; 1.0000x over previous
"""IoU metric kernel for Trainium2 (Bass/Tile), 8-core data-parallel over batch.

Problem: input [16,21,512,512] f32 logits, target [16,21,512,512] f32 0/1 masks.
  pred = argmax_C(input); per-(b,c): inter = sum(target * onehot(pred)),
  gt = sum(target), pr = sum(onehot(pred)); present = any(target) = (gt > 0).
  scores[c] = (sum_b present*inter) / (sum_b present*(gt+pr) - inter_s + eps) * counts
Returns (scores[1:], counts[1:]).

Sharding: batch 16 -> 8 cores x 2 images. Host combines per-image partials.

v6 design (fused one-pass PE reduction; DVE critical path balanced with Pool):
  - Host casts input f32 -> fp16 (argmax ties from quantization cost ~1.1e-3
    rel err, gate is 2e-2; HW-verified) and target -> fp8e4m3 (exact for 0/1).
    DMA per core: 22 MB + 11 MB = 33 MB -> ~95 us at the model's 360 GB/s.
  - Per 128-row chunk: a 7-instruction pairwise-max tree (fp16 tensor_tensor,
    2x_1p DVE mode) then one-hot planes via is_equal against a stride-0
    broadcast of the max. Pool cannot run max/is_equal TensorTensor on TRN2
    (codegen engine-check), but it CAN run subtract + tensor_scalar, so 5 of
    the 21 one-hot planes compute on Pool as (x - m) then ==0.
  - Both the target tile and the one-hot tile carry a 22nd all-ones plane
    (written once per buffer at startup). One matmul per w column,
    psum[22,22] += t_aug[:,:,w].T @ oh_aug[:,:,w], accumulates ALL THREE
    quantities at once: diag = inter, row 21 = pr (ones.T @ oh), col 21 = gt
    (t.T @ ones). No selector matmuls, no separate multiply, no reduces.
  - Schedule: target DMAs are deferred one chunk so the last input chunk
    lands ~4 us earlier; the last chunk skips the Pool offload (cross-engine
    tail) and splits its one-hot + matmuls by w-halves so PE overlaps the
    final DVE work.
  - PSUM [22,22] f32 is copied to SBUF and DMA'd out per image; host applies
    the reference formula in f64.
"""

import threading

import numpy as np

import concourse.bacc as bacc
import concourse.mybir as mybir
import concourse.tile as tile
from concourse.alu_op_type import AluOpType
from concourse.bass_utils import run_bass_kernel_spmd

F32 = mybir.dt.float32
F16 = mybir.dt.float16
F8 = mybir.dt.float8e4

B, C, H, W = 16, 21, 512, 512
NCORES = 8
BPC = B // NCORES  # images per core
P = 128
CA = C + 1  # classes + ones plane
SD = CA  # stats dim = 22
POOL_EQ = 5  # one-hot planes computed on Pool (as sub + ==0)
NXB = 3  # input-tile buffers


def build_kernel_ir(nc, bpc=BPC):
    chunks = H // P  # h-row chunks per image
    tt, eq, sub = AluOpType.max, AluOpType.is_equal, AluOpType.subtract

    inp = nc.dram_tensor("input", [bpc, C, H, W], F16, kind="ExternalInput")
    tgt = nc.dram_tensor("target", [bpc, C, H, W], F8, kind="ExternalInput")
    stats = nc.dram_tensor("stats", [bpc, SD, SD], F32, kind="ExternalOutput")

    inp_r = inp.ap().rearrange("b c (j p) w -> b j p c w", p=P)
    tgt_r = tgt.ap().rearrange("b c (j p) w -> b j p c w", p=P)
    stats_ap = stats.ap()
    nflat = bpc * chunks

    with tile.TileContext(nc) as tc:
        with tc.tile_pool(name="data", bufs=1) as dp, \
             tc.tile_pool(name="psum", bufs=1, space="PSUM") as pp:
            xb = [dp.tile([P, C, W], F16, tag=f"xb{i}", name=f"xb{i}") for i in range(NXB)]
            tb = [dp.tile([P, CA, W], F8, tag=f"tb{i}", name=f"tb{i}") for i in range(2)]
            oh = [dp.tile([P, CA, W], F16, tag=f"oh{i}", name=f"oh{i}") for i in range(2)]
            scr = [dp.tile([P, 10, W], F16, tag=f"scr{i}", name=f"scr{i}") for i in range(2)]
            m = [dp.tile([P, W], F16, tag=f"m{i}", name=f"m{i}") for i in range(2)]
            pscr = [dp.tile([P, POOL_EQ, W], F16, tag=f"pscr{i}", name=f"pscr{i}") for i in range(1)]
            res = [dp.tile([SD, SD], F32, tag=f"res{i}", name=f"res{i}") for i in range(bpc)]

            # ones planes: written once; DMA / one-hot writes never touch them
            for i in range(2):
                nc.vector.memset(tb[i][:, C, :], 1.0)
                nc.vector.memset(oh[i][:, C, :], 1.0)

            psums = [
                pp.tile([SD, SD], F32, tag=f"ps{i}", name=f"ps{i}") for i in range(bpc)
            ]

            def emit_compute(it):
                img, j = divmod(it, chunks)
                last = it == nflat - 1
                x, t, o = xb[it % NXB], tb[it % 2], oh[it % 2]
                s, mx, ps = scr[it % 2], m[it % 2], pscr[0]

                # pairwise-max tree; after op 3: s[i] = max over classes
                # {i, i+5, i+10, i+15}; class 20 merges at the end
                nc.vector.tensor_tensor(s[:, 0:5, :], x[:, 0:5, :], x[:, 5:10, :], tt)
                nc.vector.tensor_tensor(s[:, 5:10, :], x[:, 10:15, :], x[:, 15:20, :], tt)
                nc.vector.tensor_tensor(s[:, 0:5, :], s[:, 0:5, :], s[:, 5:10, :], tt)
                nc.vector.tensor_tensor(s[:, 0:2, :], s[:, 0:2, :], s[:, 2:4, :], tt)
                nc.vector.tensor_tensor(s[:, 0, :], s[:, 0, :], s[:, 1, :], tt)
                nc.vector.tensor_tensor(s[:, 0, :], s[:, 0, :], s[:, 4, :], tt)
                nc.vector.tensor_tensor(mx[:], s[:, 0, :], x[:, 20, :], tt)

                if last:
                    # keep the tail on DVE and interleave PE by w-halves
                    for (w0, w1) in ((0, W // 2), (W // 2, W)):
                        mbv = mx[:, w0:w1].unsqueeze(1)
                        nc.vector.tensor_tensor(
                            o[:, 0:C, w0:w1], x[:, 0:C, w0:w1],
                            mbv.broadcast_to((P, C, w1 - w0)), eq,
                        )
                        for g in range(w0, w1):
                            nc.tensor.matmul(
                                psums[img][:, :], t[:, :, g], o[:, :, g],
                                start=(j == 0 and g == 0),
                                stop=(j == chunks - 1 and g == W - 1),
                            )
                else:
                    ndve = C - POOL_EQ
                    mb = mx[:].unsqueeze(1)
                    nc.vector.tensor_tensor(
                        o[:, 0:ndve, :], x[:, 0:ndve, :],
                        mb.broadcast_to((P, ndve, W)), eq,
                    )
                    nc.gpsimd.tensor_tensor(
                        ps[:], x[:, ndve:C, :],
                        mb.broadcast_to((P, POOL_EQ, W)), sub,
                    )
                    nc.gpsimd.tensor_scalar(
                        out=o[:, ndve:C, :], in0=ps[:],
                        scalar1=0.0, scalar2=None, op0=eq,
                    )
                    for g in range(W):
                        nc.tensor.matmul(
                            psums[img][:, :], t[:, :, g], o[:, :, g],
                            start=(j == 0 and g == 0),
                            stop=(j == chunks - 1 and g == W - 1),
                        )
                if j == chunks - 1:
                    nc.vector.tensor_copy(res[img][:], psums[img][:])
                    nc.sync.dma_start(out=stats_ap[img], in_=res[img][:])

            # target DMAs deferred one chunk: the DMA queue runs
            # xb0,xb1,tb0,xb2,tb1,... so the last input lands earlier
            for it in range(nflat):
                img, j = divmod(it, chunks)
                nc.sync.dma_start(out=xb[it % NXB][:], in_=inp_r[img, j])
                if it > 0:
                    im1, jm1 = divmod(it - 1, chunks)
                    nc.sync.dma_start(out=tb[(it - 1) % 2][:, 0:C, :], in_=tgt_r[im1, jm1])
                    emit_compute(it - 1)
            im1, jm1 = divmod(nflat - 1, chunks)
            nc.sync.dma_start(out=tb[(nflat - 1) % 2][:, 0:C, :], in_=tgt_r[im1, jm1])
            emit_compute(nflat - 1)

    return nc


_BUILD_LOCK = threading.Lock()
_NC_CACHE = {}


def get_compiled_nc(key="full"):
    with _BUILD_LOCK:
        if key not in _NC_CACHE:
            nc = bacc.Bacc("TRN2", target_bir_lowering=False, debug=False)
            build_kernel_ir(nc)
            nc.compile()
            _NC_CACHE[key] = nc
        return _NC_CACHE[key]


def combine_stats(stats_all):
    """stats_all: [B, 22, 22] fused psum dumps -> (scores[1:], counts[1:])."""
    M = np.asarray(stats_all, dtype=np.float64)  # [B, 22, 22]
    ci = np.arange(C)
    inter = M[:, ci, ci]  # [B, C] diagonal
    pr = M[:, C, :C]      # [B, C] ones row
    gt = M[:, :C, C]      # [B, C] ones col

    present = (gt > 0).astype(np.float64)
    inter_s = (present * inter).sum(0)
    union_s = (present * (gt + pr)).sum(0) - inter_s + 1e-7
    counts = present.sum(0)
    scores = (inter_s / union_s) * counts
    return scores[1:].astype(np.float32), counts[1:].astype(np.float32)


def prep_inputs(input, target):
    import ml_dtypes

    inp = np.asarray(input, dtype=np.float32)
    tgt = np.asarray(target, dtype=np.float32)
    assert inp.shape == (B, C, H, W), inp.shape
    inp16 = np.ascontiguousarray(inp.astype(np.float16))
    tgt8 = np.ascontiguousarray(tgt.astype(ml_dtypes.float8_e4m3))
    return [
        {
            "input": inp16[i * BPC:(i + 1) * BPC],
            "target": tgt8[i * BPC:(i + 1) * BPC],
        }
        for i in range(NCORES)
    ]


def kernel(input, target):
    in_maps = prep_inputs(input, target)
    nc = get_compiled_nc()
    res = run_bass_kernel_spmd(nc, in_maps, core_ids=list(range(NCORES)))
    stats_all = np.concatenate([r["stats"] for r in res.results], axis=0)  # [B,22,22]
    return combine_stats(stats_all)


if __name__ == "__main__":
    rng = np.random.default_rng(0)
    x = rng.standard_normal((B, C, H, W), dtype=np.float32)
    t = (rng.random((B, C, H, W)) < 0.05).astype(np.float32)
    s, c = kernel(input=x, target=t)
    print("scores:", s)
    print("counts:", c)
